# revision 13
# baseline (speedup 1.0000x reference)
"""Trainium2 Bass kernel: single-head attention encoder block.

Problem: x[4, 2048, 1024]; q/k/v projections, softmax attention, output
projection, layernorm.  8 NeuronCores, SPMD.

Sharding: core c handles batch b = c // 2 and query-half h = c % 2.
Each core receives its batch's x ROTATED along the sequence axis so that
the core's 1024 query rows always occupy rows 0:1024 (attention is
permutation-invariant over keys as long as the A and x row orderings
match, so the rotation only permutes the reduction order).  This keeps
the SPMD program free of per-core constants.

Algebraic restructuring (vs the straight reference dataflow):
  * W_qk = Wq @ Wk^T and W_vo = Wv @ Wo are folded on the HOST, so the
    device never computes the K projection or separate V/O projections:
      scores = x_q W_qk x^T,   out = LN(A x W_vo).
  * LayerNorm is invariant to a positive per-row scale, so the softmax
    denominator cancels: LN(softmax(s) x Wvo) = LN(exp(s) x Wvo).  No
    denominator, no reciprocal, no normalization pass.
  * All matmul operands are bf16 (full PE rate); PSUM accumulates fp32.
    LN runs in fp32.
  * x^T is produced by the DMA XBAR transpose (2-byte dtype): zero PE
    work.  The PE only runs the four irreducible GEMM phases, which sit
    at the bf16 roofline (6.44e9 MACs/core ~= 164us at 78.6 TFLOP/s).

Per-core dataflow (P=partition dim 128):
  xr    = x rows               ([s, d] bf16, 16 tiles, DRAM read once)
  xt    = x^T                  (DMA XBAR transpose, [d, s])
  t^T   = W_qk^T @ x_q^T       ([dk, q], resident)
  S^T   = x^T.T @ t^T          ([s, q]) -> exp via ACT -> A^T (bf16)
  Z^T   = xr.T @ A^T           ([d, q]; lhsT = resident xr tiles)
  h     = Z^T.T @ W_vo         ([q, d] fp32)
  out   = layernorm(h) * gamma + beta

The LN chain is software-pipelined with a skew (engine queues are
in-order; cross-engine waits head-of-line-block), its statistics come
free from ACT accum_out during the PSUM drains, and the O phase is
interleaved with C1's second query-half so the pipeline drains under PE
work.  Engine split keeps every engine under the PE's 3.4us per q-tile:
drains+squares+sqrt on ACT, combines+normalize+beta on DVE, gamma on
GPSIMD (no PSUM port, SBUF-only ops).
"""

from contextlib import ExitStack

import numpy as np

import concourse.bass as bass
import concourse.tile as tile
from concourse import bacc, mybir
from concourse.bass_utils import run_bass_kernel_spmd

F32 = mybir.dt.float32
BF16 = mybir.dt.bfloat16
AF = mybir.ActivationFunctionType
OP = mybir.AluOpType

B = 4
S = 2048
D = 1024
NQ = 1024  # queries per core
P = 128
DT = D // P   # 8 d-tiles
ST = S // P   # 16 s-tiles
QTN = NQ // P  # 8 q-tiles
NC = 512      # matmul free-dim chunk (one fp32 PSUM bank)
QCN = NQ // NC  # 2 q-chunks
DCN = D // NC   # 2 d-chunks
N_CORES = 8
SCALE = 1.0 / np.sqrt(np.float32(D))  # 1/32
LN_EPS = 1e-5


def _keepalive(nc, tc, aps, out):
    """Read one column of each AP and DMA to out so bacc keeps the work."""
    kp = tc.alloc_tile_pool(name="keep", bufs=1, side="left")
    kt = kp.tile([P, max(len(aps), 1)], F32, tag="keep", name="keept")
    for i, ap in enumerate(aps):
        nc.vector.tensor_copy(kt[:, i:i + 1], ap[:, 0:1])
    nc.sync.dma_start(out[0:P, 0:max(len(aps), 1)], kt[:])
    kp.release()


class _NoopPool:
    def release(self):
        pass


def _emit(ctx: ExitStack, tc: tile.TileContext, io: dict, upto: str = "full",
          skip_loads: bool = False):
    nc = tc.nc
    xb = io["xb"]          # [S, D] bf16 (rolled so own queries are rows 0:NQ)
    wqk = io["wqk"]        # [D, D] bf16 = Wq @ Wk^T
    wvo = io["wvo"]        # [D, D] bf16 = Wv @ Wo
    gamma_b = io["gamma_b"]  # [P, D] f32 (broadcast)
    beta_b = io["beta_b"]
    out = io["out"]        # [NQ, D] f32
    eps_sb = io["eps_sb"]  # [P, 1] f32 (persistent, set up once per program)

    # PSUM: 8 banks.  5 rotate through S/C1/O (groups are 8-16 MMs deep,
    # so drain slack is huge); phase P gets 3 banks of its own so its
    # groups only WAR against the PREVIOUS iteration's phase P (long
    # drained) instead of its last O drains — the next iteration's first
    # matmuls start the moment the PE queue empties, and P's 3-deep
    # rotation rides out the boundary DVE/ACT queue clog.
    ps_mm = ctx.enter_context(tc.tile_pool(name="ps_mm", bufs=5, space="PSUM"))
    ps_p = ctx.enter_context(tc.tile_pool(name="ps_p", bufs=3, space="PSUM"))

    # Right stack, bottom-up by lifetime: zt+wvo+LN pools (die at end), at
    # (end of C1), tT (end of S).  The LN pools (h/hsq/stat) live on the
    # RIGHT, under at/tT, so the left-stack xtb/wqk regions have no later
    # aliasing alloc: the NEXT iteration's wqk/x^T DMAs are WAR-blocked
    # only on this iteration's phase-P/S reads (not on the LN tail), so
    # they prefetch ~a full phase early and the PE never idles at the
    # iteration boundary.
    pre = io.get("pre")
    zt_pool = tc.alloc_tile_pool(name="ztp", bufs=1, side="right")
    zt_sb = [zt_pool.tile([P, NQ], BF16, tag=f"zt{d}", name=f"zt{d}") for d in range(DT)]
    if pre is None:
        wvo_pool = tc.alloc_tile_pool(name="wvop", bufs=1, side="right")
        wvo_sb = [wvo_pool.tile([P, D], BF16, tag=f"wvo{v}", name=f"wvo{v}") for v in range(DT)]
    else:
        wvo_pool, wvo_sb = _NoopPool(), pre["wvo_sb"]
    h_pool = tc.alloc_tile_pool(name="hp", bufs=4, side="right")
    hsq_pool = tc.alloc_tile_pool(name="hsqp", bufs=1, side="right")
    stat_pool = tc.alloc_tile_pool(name="statp", bufs=4, side="right")
    at_pool = tc.alloc_tile_pool(name="atp", bufs=1, side="right")
    at_sb = [at_pool.tile([P, NQ], BF16, tag=f"at{st}", name=f"at{st}") for st in range(ST)]
    tT_pool = tc.alloc_tile_pool(name="tTp", bufs=1, side="right")
    tT_sb = [tT_pool.tile([P, NQ], BF16, tag=f"tT{k}", name=f"tT{k}") for k in range(DT)]

    # Left stack: xrow (resident through C1; C1's lhsT), xtb (through S),
    # wqk (through P).
    if pre is None:
        xrow_pool = tc.alloc_tile_pool(name="xrow", bufs=1, side="left")
        xr_sb = [xrow_pool.tile([P, D], BF16, tag=f"xr{st}", name=f"xr{st}") for st in range(ST)]
        # gb sits under xtb on the left stack (it outlives xtb); its DMA is
        # issued later, right before phase S.
        gb_pool = tc.alloc_tile_pool(name="gbp", bufs=1, side="left")
        gamma_sb = gb_pool.tile([P, D], F32, tag="gamma", name="gamma_sb")
        beta_sb = gb_pool.tile([P, D], F32, tag="beta", name="beta_sb")
        xtb_pool = tc.alloc_tile_pool(name="xtb", bufs=1, side="left")
        xtb = [xtb_pool.tile([P, S], BF16, tag=f"xtb{d}", name=f"xtb{d}") for d in range(DT)]
        wqk_pool = tc.alloc_tile_pool(name="wqkp", bufs=1, side="left")
        wqk_sb = [wqk_pool.tile([P, D], BF16, tag=f"wqk{d}", name=f"wqk{d}") for d in range(DT)]
    else:
        xrow_pool = gb_pool = xtb_pool = wqk_pool = _NoopPool()
        xr_sb, xtb, wqk_sb = pre["xr_sb"], pre["xtb"], pre["wqk_sb"]
        gamma_sb, beta_sb = pre["gamma_sb"], pre["beta_sb"]
        skip_loads = True

    # x^T entirely via the DMA XBAR transpose (2-byte dtype): zero PE
    # work, no PSUM traffic, no drains.  One dma_start per (d, chunk) so
    # the transposes spread across DMA rings.  In steady state (NREP
    # back-to-back iterations) all of these prefetch during the previous
    # iteration's compute.
    def _transpose_chunk(sc):
        for d in range(DT):
            nc.sync.dma_start(
                xtb[d][:, sc * NC:(sc + 1) * NC],
                xb[sc * NC:(sc + 1) * NC, d * P:(d + 1) * P],
                transpose=True,
            )

    if not skip_loads:
        _transpose_chunk(0)
        for d in range(DT):
            nc.sync.dma_start(wqk_sb[d][:], wqk[d * P:(d + 1) * P, :])
        _transpose_chunk(1)
        _transpose_chunk(2)
        _transpose_chunk(3)
        for st in range(ST):
            nc.sync.dma_start(xr_sb[st][:], xb[st * P:(st + 1) * P, :])

    def _tproj(qc):
        # t^T[dk, q] = sum_d W_qk[d, dk] x^T[d, q] for q in this chunk.
        # Drain-engine choice: at an iteration boundary the ACT queue still
        # holds the previous iteration's LN tail (~5us), so the first
        # chunk's early drains go to DVE (whose tail clears in ~2us);
        # later groups alternate to spread load.
        for dk in range(DT):
            ps = ps_p.tile([P, NC], F32, tag="mm", name=f"psP{dk}_{qc}")
            for d in range(DT):
                nc.tensor.matmul(
                    ps[:],
                    wqk_sb[d][:, dk * P:(dk + 1) * P],
                    xtb[d][:, qc * NC:(qc + 1) * NC],
                    start=(d == 0),
                    stop=(d == DT - 1),
                )
            if (dk < 4 and qc == 0) or (qc != 0 and dk % 2 == 0):
                nc.vector.tensor_copy(tT_sb[dk][:, qc * NC:(qc + 1) * NC], ps[:])
            else:
                nc.scalar.copy(tT_sb[dk][:, qc * NC:(qc + 1) * NC], ps[:])

    # ---- Phase P: t^T = W_qk^T @ x_q^T ----
    _tproj(0)
    _tproj(1)
    wqk_pool.release()

    if upto == "P":
        _keepalive(
            nc, tc,
            [t[:, 0:1] for t in xtb] + [t[:, 0:1] for t in tT_sb]
            + [t[:, 0:1] for t in xr_sb], out)
        xtb_pool.release()
        gb_pool.release()
        xrow_pool.release()
        tT_pool.release()
        at_pool.release()
        stat_pool.release()
        hsq_pool.release()
        h_pool.release()
        wvo_pool.release()
        zt_pool.release()
        return

    # Prefetch for later phases; DMA engines are idle during S.
    if not skip_loads:
        for v in range(DT):
            nc.sync.dma_start(wvo_sb[v][:], wvo[v * P:(v + 1) * P, :])
        nc.sync.dma_start(gamma_sb[:], gamma_b[:])
        nc.sync.dma_start(beta_sb[:], beta_b[:])

    # ---- Phase S: scores^T -> exp (UNNORMALIZED attention weights) ----
    # softmax's denominator is skipped entirely: LayerNorm at the end is
    # invariant to the positive per-row scale it would apply.  Max-
    # subtraction is unnecessary (scores are O(1) by construction).
    for qc in range(QCN):
        for st in range(ST):
            ps = ps_mm.tile([P, NC], F32, tag="mm", name=f"psS{qc}_{st}")
            for dk in range(DT):
                nc.tensor.matmul(
                    ps[:],
                    xtb[dk][:, st * P:(st + 1) * P],
                    tT_sb[dk][:, qc * NC:(qc + 1) * NC],
                    start=(dk == 0),
                    stop=(dk == DT - 1),
                )
            nc.scalar.activation(
                at_sb[st][:, qc * NC:(qc + 1) * NC], ps[:], AF.Exp,
                scale=float(SCALE),
            )
    tT_pool.release()
    xtb_pool.release()

    if upto == "S":
        _keepalive(
            nc, tc,
            [t[:, 0:1] for t in at_sb] + [t[:, 0:1] for t in xr_sb], out)
        gb_pool.release()
        xrow_pool.release()
        at_pool.release()
        stat_pool.release()
        hsq_pool.release()
        h_pool.release()
        wvo_pool.release()
        zt_pool.release()
        return

    # ---- Phase C1: Z^T = x^T @ A^T  ([d, q]; lhsT = resident xr tiles) ----
    def _c1_group(d, qc):
        ps = ps_mm.tile([P, NC], F32, tag="mm", name=f"psZ{d}_{qc}")
        for st in range(ST):
            nc.tensor.matmul(
                ps[:],
                xr_sb[st][:, d * P:(d + 1) * P],
                at_sb[st][:, qc * NC:(qc + 1) * NC],
                start=(st == 0),
                stop=(st == ST - 1),
            )
        if d % 2 == 0:
            nc.vector.tensor_copy(zt_sb[d][:, qc * NC:(qc + 1) * NC], ps[:])
        else:
            nc.scalar.copy(zt_sb[d][:, qc * NC:(qc + 1) * NC], ps[:])

    for d in range(DT):
        _c1_group(d, 0)

    if upto == "C1":
        for d in range(DT):
            _c1_group(d, 1)
        at_pool.release()
        _keepalive(
            nc, tc,
            [t[:, 0:1] for t in zt_sb] + [t[:, 0:1] for t in wvo_sb], out)
        gb_pool.release()
        xrow_pool.release()
        stat_pool.release()
        hsq_pool.release()
        h_pool.release()
        wvo_pool.release()
        zt_pool.release()
        return

    # ---- Phase O: h = Z^T.T @ W_vo, layernorm, store ----
    # LN statistics come for free from ACT accum_out: the PSUM drains
    # produce per-row sums, and two extra ACT Square passes (reading the
    # same PSUM bank) produce per-row sums of squares.  No bn_stats on
    # DVE: it only runs tiny [P,1] combines + normalize + beta-add, so
    # every engine stays under the PE's 3.4us per q-tile.
    # Engine queues are IN-ORDER: an instruction waiting on a cross-engine
    # dependency head-of-line-blocks everything behind it.  The LN chain
    # has ~6 cross-engine hops, so it is staged as a skewed pipeline where
    # every instruction emitted in iteration i depends only on results
    # from iteration i-1 or older (except gamma, which trails its own
    # iteration's ts on the otherwise-idle GPSIMD).
    #   iter i:  mm+drain+sums(i) | combine+sqrt(i-1) | ts(i-2), gamma(i-2)
    #            | beta+store(i-3)
    # The normalize/gamma/beta chain runs IN PLACE over h (each stage is a
    # full pipeline step apart, so the RMW chain adds no new stalls); this
    # keeps the right-stack LN footprint small enough to coexist with at.
    RD = float(1.0 / D)
    st_h = {}
    st_sums = {}
    st_mu = {}
    st_rstd = {}

    def _stage_mm(qt):
        h = h_pool.tile([P, D], F32, tag="h", name=f"h{qt}")
        hsq = hsq_pool.tile([P, NC], F32, tag="hsq", name=f"hsq{qt}")
        sums = stat_pool.tile([P, 4], F32, tag="sums", name=f"sums{qt}")
        for dc in range(DCN):
            ps = ps_mm.tile([P, NC], F32, tag="mm", name=f"psO{qt}_{dc}")
            for v in range(DT):
                nc.tensor.matmul(
                    ps[:],
                    zt_sb[v][:, qt * P:(qt + 1) * P],
                    wvo_sb[v][:, dc * NC:(dc + 1) * NC],
                    start=(v == 0),
                    stop=(v == DT - 1),
                )
            nc.scalar.activation(
                h[:, dc * NC:(dc + 1) * NC], ps[:], AF.Copy,
                accum_out=sums[:, dc:dc + 1],
            )
            nc.scalar.activation(
                hsq[:], ps[:], AF.Square,
                accum_out=sums[:, 2 + dc:3 + dc],
            )
        st_h[qt] = h
        st_sums[qt] = sums

    def _stage_stats(qt):
        # mu = (s0+s1)/D; var = (q0+q1)/D - mu^2; rstd_pre = sqrt(var+eps)
        sums = st_sums[qt]
        mu = stat_pool.tile([P, 1], F32, tag="mu", name=f"mu{qt}")
        msum = stat_pool.tile([P, 1], F32, tag="msum", name=f"msum{qt}")
        nc.vector.tensor_tensor(msum[:], sums[:, 0:1], sums[:, 1:2], OP.add)
        nc.vector.tensor_scalar(
            out=mu[:], in0=msum[:], scalar1=RD, scalar2=None, op0=OP.mult,
        )
        m2 = stat_pool.tile([P, 1], F32, tag="m2", name=f"m2{qt}")
        nc.vector.tensor_tensor(m2[:], mu[:], mu[:], OP.mult)
        qsum = stat_pool.tile([P, 1], F32, tag="qsum", name=f"qsum{qt}")
        nc.vector.tensor_tensor(qsum[:], sums[:, 2:3], sums[:, 3:4], OP.add)
        var = stat_pool.tile([P, 1], F32, tag="var", name=f"var{qt}")
        nc.vector.scalar_tensor_tensor(
            var[:], qsum[:], RD, m2[:], OP.mult, OP.subtract,
        )
        rstd = stat_pool.tile([P, 1], F32, tag="rstd", name=f"rstd{qt}")
        nc.scalar.activation(rstd[:], var[:], AF.Sqrt, bias=eps_sb[:], scale=1.0)
        st_mu[qt] = mu
        st_rstd[qt] = rstd

    def _stage_norm(qt):
        # Column-halved ts/gamma: halves pipeline across DVE and GPSIMD,
        # halving the serial chain latency (matters for the last tiles).
        # Both write h in place.
        rstd = st_rstd[qt]
        nc.vector.reciprocal(rstd[:], rstd[:])
        h = st_h[qt]
        for c in range(2):
            cs = slice(c * NC, (c + 1) * NC)
            nc.vector.tensor_scalar(
                out=h[:, cs],
                in0=h[:, cs],
                scalar1=st_mu[qt][:],
                scalar2=rstd[:],
                op0=OP.subtract,
                op1=OP.mult,
            )
            nc.gpsimd.tensor_tensor(h[:, cs], h[:, cs], gamma_sb[:, cs], OP.mult)

    def _stage_store(qt):
        h = st_h[qt]
        for c in range(2):
            cs = slice(c * NC, (c + 1) * NC)
            nc.vector.tensor_tensor(h[:, cs], h[:, cs], beta_sb[:, cs], OP.add)
            nc.sync.dma_start(out[qt * P:(qt + 1) * P, cs], h[:, cs])

    # Interleave: O's first q-half runs right after C1's qc0 columns are
    # ready; C1's qc1 groups then cover the tail of those LN chains, so
    # only the last q-half's pipeline drains after the final matmul.
    def _o_batch(q0):
        for i in range(q0, q0 + 4):
            _stage_mm(i)
            if i - 1 >= q0:
                _stage_stats(i - 1)
            if i - 2 >= q0:
                _stage_norm(i - 2)
            if i - 3 >= q0:
                _stage_store(i - 3)

    _o_batch(0)
    leftovers = [lambda: _stage_stats(3), lambda: _stage_norm(2),
                 lambda: _stage_store(1), lambda: _stage_norm(3),
                 lambda: _stage_store(2), lambda: _stage_store(3)]
    for d in range(DT):
        _c1_group(d, 1)
        if leftovers:
            leftovers.pop(0)()
    for f in leftovers:
        f()
    at_pool.release()
    _o_batch(4)
    _stage_stats(7)
    _stage_norm(6)
    _stage_store(5)
    _stage_norm(7)
    _stage_store(6)
    _stage_store(7)
    stat_pool.release()
    hsq_pool.release()
    h_pool.release()
    gb_pool.release()
    xrow_pool.release()
    wvo_pool.release()
    zt_pool.release()


_PROGS: dict = {}


def _build_program(n_iters: int = 1, upto: str = "full",
                   loads_every_iter: bool = True):
    key = (n_iters, upto, loads_every_iter)
    if key not in _PROGS:
        nc = bacc.Bacc(
            "TRN2",
            target_bir_lowering=False,
            debug=False,
            enable_asserts=False,
            num_devices=N_CORES,
        )
        io = {
            "xb": nc.dram_tensor("xb", [S, D], BF16, kind="ExternalInput").ap(),
            "wqk": nc.dram_tensor("wqk", [D, D], BF16, kind="ExternalInput").ap(),
            "wvo": nc.dram_tensor("wvo", [D, D], BF16, kind="ExternalInput").ap(),
            "gamma_b": nc.dram_tensor("gamma_b", [P, D], F32, kind="ExternalInput").ap(),
            "beta_b": nc.dram_tensor("beta_b", [P, D], F32, kind="ExternalInput").ap(),
            "out": nc.dram_tensor("out", [NQ, D], F32, kind="ExternalOutput").ap(),
        }
        with tile.TileContext(nc) as tc:
            # Persistent constants: eps + ACT function-table pre-warm (Exp
            # for S, Sqrt/Square for O), once per program rather than per
            # iteration.
            const = tc.alloc_tile_pool(name="const", bufs=1, side="left")
            eps_sb = const.tile([P, 1], F32, tag="eps")
            nc.vector.memset(eps_sb[:], LN_EPS)
            warm = const.tile([P, 1], F32, tag="actwarm")
            nc.scalar.activation(warm[:], eps_sb[:], AF.Exp, scale=1.0)
            nc.scalar.activation(warm[:], eps_sb[:], AF.Sqrt, scale=1.0)
            nc.scalar.activation(warm[:], eps_sb[:], AF.Square, scale=1.0)
            io["eps_sb"] = eps_sb
            pre_pools = []
            if not loads_every_iter:
                # Diagnostic mode: inputs resident in SBUF, loaded once.
                nc_ = nc
                xrow_pool = tc.alloc_tile_pool(name="xrow", bufs=1, side="left")
                xr_sb = [xrow_pool.tile([P, D], BF16, tag=f"xr{st}", name=f"xr{st}") for st in range(ST)]
                gb_pool = tc.alloc_tile_pool(name="gbp", bufs=1, side="left")
                gamma_sb = gb_pool.tile([P, D], F32, tag="gamma", name="gamma_sb")
                beta_sb = gb_pool.tile([P, D], F32, tag="beta", name="beta_sb")
                xtb_pool = tc.alloc_tile_pool(name="xtb", bufs=1, side="left")
                xtb = [xtb_pool.tile([P, S], BF16, tag=f"xtb{d}", name=f"xtb{d}") for d in range(DT)]
                wqk_pool = tc.alloc_tile_pool(name="wqkp", bufs=1, side="left")
                wqk_sb = [wqk_pool.tile([P, D], BF16, tag=f"wqk{d}", name=f"wqk{d}") for d in range(DT)]
                wvo_pool = tc.alloc_tile_pool(name="wvop", bufs=1, side="right")
                wvo_sb = [wvo_pool.tile([P, D], BF16, tag=f"wvo{v}", name=f"wvo{v}") for v in range(DT)]
                pre_pools = [wvo_pool, wqk_pool, xtb_pool, gb_pool, xrow_pool]
                for sc in range(4):
                    for d in range(DT):
                        nc_.sync.dma_start(
                            xtb[d][:, sc * NC:(sc + 1) * NC],
                            io["xb"][sc * NC:(sc + 1) * NC, d * P:(d + 1) * P],
                            transpose=True,
                        )
                for d in range(DT):
                    nc_.sync.dma_start(wqk_sb[d][:], io["wqk"][d * P:(d + 1) * P, :])
                for st in range(ST):
                    nc_.sync.dma_start(xr_sb[st][:], io["xb"][st * P:(st + 1) * P, :])
                for v in range(DT):
                    nc_.sync.dma_start(wvo_sb[v][:], io["wvo"][v * P:(v + 1) * P, :])
                nc_.sync.dma_start(gamma_sb[:], io["gamma_b"][:])
                nc_.sync.dma_start(beta_sb[:], io["beta_b"][:])
                io["pre"] = {
                    "xr_sb": xr_sb, "xtb": xtb, "wqk_sb": wqk_sb,
                    "wvo_sb": wvo_sb, "gamma_sb": gamma_sb, "beta_sb": beta_sb,
                }
            for i in range(n_iters):
                with ExitStack() as ctx:
                    _emit(ctx, tc, io, upto)
            io.pop("pre", None)
            for p in pre_pools:
                p.release()
            const.release()
        nc.compile()
        _PROGS[key] = nc
    return _PROGS[key]


LAST_RESULTS = None


def _host_inputs(x, Wq, Wk, Wv, Wo, ln2_gamma, ln2_beta):
    """Fold weights and cast; returns the per-core input maps."""
    import ml_dtypes

    bf16 = ml_dtypes.bfloat16
    x = np.asarray(x, dtype=np.float32)
    Wq = np.asarray(Wq, dtype=np.float64)
    Wk = np.asarray(Wk, dtype=np.float64)
    Wv = np.asarray(Wv, dtype=np.float64)
    Wo = np.asarray(Wo, dtype=np.float64)
    wqk = np.ascontiguousarray((Wq @ Wk.T).astype(bf16))
    wvo = np.ascontiguousarray((Wv @ Wo).astype(bf16))
    gamma_b = np.ascontiguousarray(
        np.broadcast_to(np.asarray(ln2_gamma, dtype=np.float32), (P, D))
    )
    beta_b = np.ascontiguousarray(
        np.broadcast_to(np.asarray(ln2_beta, dtype=np.float32), (P, D))
    )
    x16 = x.astype(bf16)
    in_maps = []
    for c in range(N_CORES):
        b, h = c // 2, c % 2
        # Rotate so this core's query rows are rows 0:NQ.
        xb = np.ascontiguousarray(np.roll(x16[b], -h * NQ, axis=0))
        in_maps.append(
            {
                "xb": xb,
                "wqk": wqk,
                "wvo": wvo,
                "gamma_b": gamma_b,
                "beta_b": beta_b,
            }
        )
    return in_maps


def kernel(x, Wq, Wk, Wv, Wo, ln2_gamma, ln2_beta):
    global LAST_RESULTS
    in_maps = _host_inputs(x, Wq, Wk, Wv, Wo, ln2_gamma, ln2_beta)
    nc = _build_program()
    res = run_bass_kernel_spmd(nc, in_maps, list(range(N_CORES)))
    LAST_RESULTS = res
    out = np.empty((B, S, D), dtype=np.float32)
    for c in range(N_CORES):
        b, h = c // 2, c % 2
        out[b, h * NQ:(h + 1) * NQ] = res.results[c]["out"]
    return out



# revision 14
# speedup vs baseline: 1.2701x; 1.2701x over previous
"""Trainium2 Bass kernel: single-head attention encoder block.

Problem: x[4, 2048, 1024]; q/k/v projections, softmax attention, output
projection, layernorm.  8 NeuronCores, SPMD.

Sharding: core c handles batch b = c // 2 and query-half h = c % 2.
Each core receives its batch's x ROTATED along the sequence axis so that
the core's 1024 query rows always occupy rows 0:1024 (attention is
permutation-invariant over keys as long as the A and x row orderings
match, so the rotation only permutes the reduction order).  This keeps
the SPMD program free of per-core constants.

Algebraic restructuring (vs the straight reference dataflow):
  * W_qk = Wq @ Wk^T and W_vo = Wv @ Wo are folded on the HOST, so the
    device never computes the K projection or separate V/O projections:
      scores = x_q W_qk x^T,   out = LN(A x W_vo).
  * LayerNorm is invariant to a positive per-row scale, so the softmax
    denominator cancels: LN(softmax(s) x Wvo) = LN(exp(s) x Wvo).  No
    denominator, no reciprocal, no normalization pass.
  * All matmul operands are bf16 (full PE rate); PSUM accumulates fp32.
    LN runs in fp32.
  * x^T is produced by the DMA XBAR transpose (2-byte dtype): zero PE
    work.  The PE only runs the four irreducible GEMM phases, which sit
    at the bf16 roofline (6.44e9 MACs/core ~= 164us at 78.6 TFLOP/s).

Per-core dataflow (P=partition dim 128):
  xr    = x rows               ([s, d] bf16, 16 tiles, DRAM read once)
  xt    = x^T                  (DMA XBAR transpose, [d, s])
  t^T   = W_qk^T @ x_q^T       ([dk, q], resident)
  S^T   = x^T.T @ t^T          ([s, q]) -> exp via ACT -> A^T (bf16)
  Z^T   = xr.T @ A^T           ([d, q]; lhsT = resident xr tiles)
  h     = Z^T.T @ W_vo         ([q, d] fp32)
  out   = layernorm(h) * gamma + beta

The LN chain is software-pipelined with a skew (engine queues are
in-order; cross-engine waits head-of-line-block), its statistics come
free from ACT accum_out during the PSUM drains, and the O phase is
interleaved with C1's second query-half so the pipeline drains under PE
work.  Engine split keeps every engine under the PE's 3.4us per q-tile:
drains+squares+sqrt on ACT, combines+normalize+beta on DVE, gamma on
GPSIMD (no PSUM port, SBUF-only ops).  normalize/gamma/beta run IN
PLACE over h.

Cross-iteration steady state (timeline-sim verified gapless):
  * The LN pools live on the RIGHT stack (under at/tT), so the left
    stack (xr, gamma/beta, x^T, wqk) has no aliasing realloc after its
    phase ends: iteration i+1's wqk DMA is WAR-blocked only on iteration
    i's phase P, and its x^T transposes only on phase S — both prefetch
    a full phase (or more) ahead of use.
  * PSUM is split 5+3: S/C1/O rotate 5 banks (8-16-MM groups, huge
    drain slack); phase P owns 3, so iteration i+1's first matmuls WAR
    against iteration i's phase P (long drained), never its LN tail.
  * Phase P's first-chunk drains go to DVE: at the boundary the ACT
    queue still holds the previous LN tail (~5us).
"""

from contextlib import ExitStack

import numpy as np

import concourse.bass as bass
import concourse.tile as tile
from concourse import bacc, mybir
from concourse.bass_utils import run_bass_kernel_spmd

F32 = mybir.dt.float32
BF16 = mybir.dt.bfloat16
AF = mybir.ActivationFunctionType
OP = mybir.AluOpType

B = 4
S = 2048
D = 1024
NQ = 1024  # queries per core
P = 128
DT = D // P   # 8 d-tiles
ST = S // P   # 16 s-tiles
QTN = NQ // P  # 8 q-tiles
NC = 512      # matmul free-dim chunk (one fp32 PSUM bank)
QCN = NQ // NC  # 2 q-chunks
DCN = D // NC   # 2 d-chunks
N_CORES = 8
SCALE = 1.0 / np.sqrt(np.float32(D))  # 1/32
LN_EPS = 1e-5


def _keepalive(nc, tc, aps, out):
    """Read one column of each AP and DMA to out so bacc keeps the work."""
    kp = tc.alloc_tile_pool(name="keep", bufs=1, side="left")
    kt = kp.tile([P, max(len(aps), 1)], F32, tag="keep", name="keept")
    for i, ap in enumerate(aps):
        nc.vector.tensor_copy(kt[:, i:i + 1], ap[:, 0:1])
    nc.sync.dma_start(out[0:P, 0:max(len(aps), 1)], kt[:])
    kp.release()


class _NoopPool:
    def release(self):
        pass


def _emit(ctx: ExitStack, tc: tile.TileContext, io: dict, upto: str = "full",
          skip_loads: bool = False):
    nc = tc.nc
    xb = io["xb"]          # [S, D] bf16 (rolled so own queries are rows 0:NQ)
    wqk = io["wqk"]        # [D, D] bf16 = Wq @ Wk^T
    wvo = io["wvo"]        # [D, D] bf16 = Wv @ Wo
    gamma_b = io["gamma_b"]  # [P, D] f32 (broadcast)
    beta_b = io["beta_b"]
    out = io["out"]        # [NQ, D] f32
    eps_sb = io["eps_sb"]  # [P, 1] f32 (persistent, set up once per program)

    # PSUM: 8 banks.  5 rotate through S/C1/O (groups are 8-16 MMs deep,
    # so drain slack is huge); phase P gets 3 banks of its own so its
    # groups only WAR against the PREVIOUS iteration's phase P (long
    # drained) instead of its last O drains — the next iteration's first
    # matmuls start the moment the PE queue empties, and P's 3-deep
    # rotation rides out the boundary DVE/ACT queue clog.
    ps_mm = ctx.enter_context(tc.tile_pool(name="ps_mm", bufs=5, space="PSUM"))
    ps_p = ctx.enter_context(tc.tile_pool(name="ps_p", bufs=3, space="PSUM"))

    # Right stack, bottom-up by lifetime: zt+wvo+LN pools (die at end), at
    # (end of C1), tT (end of S).  The LN pools (h/hsq/stat) live on the
    # RIGHT, under at/tT, so the left-stack xtb/wqk regions have no later
    # aliasing alloc: the NEXT iteration's wqk/x^T DMAs are WAR-blocked
    # only on this iteration's phase-P/S reads (not on the LN tail), so
    # they prefetch ~a full phase early and the PE never idles at the
    # iteration boundary.
    pre = io.get("pre")
    zt_pool = tc.alloc_tile_pool(name="ztp", bufs=1, side="right")
    zt_sb = [zt_pool.tile([P, NQ], BF16, tag=f"zt{d}", name=f"zt{d}") for d in range(DT)]
    if pre is None:
        wvo_pool = tc.alloc_tile_pool(name="wvop", bufs=1, side="right")
        wvo_sb = [wvo_pool.tile([P, D], BF16, tag=f"wvo{v}", name=f"wvo{v}") for v in range(DT)]
    else:
        wvo_pool, wvo_sb = _NoopPool(), pre["wvo_sb"]
    h_pool = tc.alloc_tile_pool(name="hp", bufs=4, side="right")
    hsq_pool = tc.alloc_tile_pool(name="hsqp", bufs=1, side="right")
    stat_pool = tc.alloc_tile_pool(name="statp", bufs=4, side="right")
    at_pool = tc.alloc_tile_pool(name="atp", bufs=1, side="right")
    at_sb = [at_pool.tile([P, NQ], BF16, tag=f"at{st}", name=f"at{st}") for st in range(ST)]
    tT_pool = tc.alloc_tile_pool(name="tTp", bufs=1, side="right")
    tT_sb = [tT_pool.tile([P, NQ], BF16, tag=f"tT{k}", name=f"tT{k}") for k in range(DT)]

    # Left stack: xrow (resident through C1; C1's lhsT), xtb (through S),
    # wqk (through P).
    if pre is None:
        xrow_pool = tc.alloc_tile_pool(name="xrow", bufs=1, side="left")
        xr_sb = [xrow_pool.tile([P, D], BF16, tag=f"xr{st}", name=f"xr{st}") for st in range(ST)]
        # gb sits under xtb on the left stack (it outlives xtb); its DMA is
        # issued later, right before phase S.
        gb_pool = tc.alloc_tile_pool(name="gbp", bufs=1, side="left")
        gamma_sb = gb_pool.tile([P, D], F32, tag="gamma", name="gamma_sb")
        beta_sb = gb_pool.tile([P, D], F32, tag="beta", name="beta_sb")
        xtb_pool = tc.alloc_tile_pool(name="xtb", bufs=1, side="left")
        xtb = [xtb_pool.tile([P, S], BF16, tag=f"xtb{d}", name=f"xtb{d}") for d in range(DT)]
        wqk_pool = tc.alloc_tile_pool(name="wqkp", bufs=1, side="left")
        wqk_sb = [wqk_pool.tile([P, D], BF16, tag=f"wqk{d}", name=f"wqk{d}") for d in range(DT)]
    else:
        xrow_pool = gb_pool = xtb_pool = wqk_pool = _NoopPool()
        xr_sb, xtb, wqk_sb = pre["xr_sb"], pre["xtb"], pre["wqk_sb"]
        gamma_sb, beta_sb = pre["gamma_sb"], pre["beta_sb"]
        skip_loads = True

    # x^T entirely via the DMA XBAR transpose (2-byte dtype): zero PE
    # work, no PSUM traffic, no drains.  One dma_start per (d, chunk) so
    # the transposes spread across DMA rings.  In steady state (NREP
    # back-to-back iterations) all of these prefetch during the previous
    # iteration's compute.
    def _transpose_chunk(sc):
        for d in range(DT):
            nc.sync.dma_start(
                xtb[d][:, sc * NC:(sc + 1) * NC],
                xb[sc * NC:(sc + 1) * NC, d * P:(d + 1) * P],
                transpose=True,
            )

    if not skip_loads:
        _transpose_chunk(0)
        for d in range(DT):
            nc.sync.dma_start(wqk_sb[d][:], wqk[d * P:(d + 1) * P, :])
        _transpose_chunk(1)
        _transpose_chunk(2)
        _transpose_chunk(3)
        for st in range(ST):
            nc.sync.dma_start(xr_sb[st][:], xb[st * P:(st + 1) * P, :])

    def _tproj(qc):
        # t^T[dk, q] = sum_d W_qk[d, dk] x^T[d, q] for q in this chunk.
        # Drain-engine choice: at an iteration boundary the ACT queue still
        # holds the previous iteration's LN tail (~5us), so the first
        # chunk's early drains go to DVE (whose tail clears in ~2us);
        # later groups alternate to spread load.
        for dk in range(DT):
            ps = ps_p.tile([P, NC], F32, tag="mm", name=f"psP{dk}_{qc}")
            for d in range(DT):
                nc.tensor.matmul(
                    ps[:],
                    wqk_sb[d][:, dk * P:(dk + 1) * P],
                    xtb[d][:, qc * NC:(qc + 1) * NC],
                    start=(d == 0),
                    stop=(d == DT - 1),
                )
            if (dk < 4 and qc == 0) or (qc != 0 and dk % 2 == 0):
                nc.vector.tensor_copy(tT_sb[dk][:, qc * NC:(qc + 1) * NC], ps[:])
            else:
                nc.scalar.copy(tT_sb[dk][:, qc * NC:(qc + 1) * NC], ps[:])

    # ---- Phase P: t^T = W_qk^T @ x_q^T ----
    _tproj(0)
    _tproj(1)
    wqk_pool.release()

    if upto == "P":
        _keepalive(
            nc, tc,
            [t[:, 0:1] for t in xtb] + [t[:, 0:1] for t in tT_sb]
            + [t[:, 0:1] for t in xr_sb], out)
        xtb_pool.release()
        gb_pool.release()
        xrow_pool.release()
        tT_pool.release()
        at_pool.release()
        stat_pool.release()
        hsq_pool.release()
        h_pool.release()
        wvo_pool.release()
        zt_pool.release()
        return

    # Prefetch for later phases; DMA engines are idle during S.
    if not skip_loads:
        for v in range(DT):
            nc.sync.dma_start(wvo_sb[v][:], wvo[v * P:(v + 1) * P, :])
        nc.sync.dma_start(gamma_sb[:], gamma_b[:])
        nc.sync.dma_start(beta_sb[:], beta_b[:])

    # ---- Phase S: scores^T -> exp (UNNORMALIZED attention weights) ----
    # softmax's denominator is skipped entirely: LayerNorm at the end is
    # invariant to the positive per-row scale it would apply.  Max-
    # subtraction is unnecessary (scores are O(1) by construction).
    for qc in range(QCN):
        for st in range(ST):
            ps = ps_mm.tile([P, NC], F32, tag="mm", name=f"psS{qc}_{st}")
            for dk in range(DT):
                nc.tensor.matmul(
                    ps[:],
                    xtb[dk][:, st * P:(st + 1) * P],
                    tT_sb[dk][:, qc * NC:(qc + 1) * NC],
                    start=(dk == 0),
                    stop=(dk == DT - 1),
                )
            nc.scalar.activation(
                at_sb[st][:, qc * NC:(qc + 1) * NC], ps[:], AF.Exp,
                scale=float(SCALE),
            )
    tT_pool.release()
    xtb_pool.release()

    if upto == "S":
        _keepalive(
            nc, tc,
            [t[:, 0:1] for t in at_sb] + [t[:, 0:1] for t in xr_sb], out)
        gb_pool.release()
        xrow_pool.release()
        at_pool.release()
        stat_pool.release()
        hsq_pool.release()
        h_pool.release()
        wvo_pool.release()
        zt_pool.release()
        return

    # ---- Phase C1: Z^T = x^T @ A^T  ([d, q]; lhsT = resident xr tiles) ----
    def _c1_group(d, qc):
        ps = ps_mm.tile([P, NC], F32, tag="mm", name=f"psZ{d}_{qc}")
        for st in range(ST):
            nc.tensor.matmul(
                ps[:],
                xr_sb[st][:, d * P:(d + 1) * P],
                at_sb[st][:, qc * NC:(qc + 1) * NC],
                start=(st == 0),
                stop=(st == ST - 1),
            )
        if d % 2 == 0:
            nc.vector.tensor_copy(zt_sb[d][:, qc * NC:(qc + 1) * NC], ps[:])
        else:
            nc.scalar.copy(zt_sb[d][:, qc * NC:(qc + 1) * NC], ps[:])

    for d in range(DT):
        _c1_group(d, 0)

    if upto == "C1":
        for d in range(DT):
            _c1_group(d, 1)
        at_pool.release()
        _keepalive(
            nc, tc,
            [t[:, 0:1] for t in zt_sb] + [t[:, 0:1] for t in wvo_sb], out)
        gb_pool.release()
        xrow_pool.release()
        stat_pool.release()
        hsq_pool.release()
        h_pool.release()
        wvo_pool.release()
        zt_pool.release()
        return

    # ---- Phase O: h = Z^T.T @ W_vo, layernorm, store ----
    # LN statistics come for free from ACT accum_out: the PSUM drains
    # produce per-row sums, and two extra ACT Square passes (reading the
    # same PSUM bank) produce per-row sums of squares.  No bn_stats on
    # DVE: it only runs tiny [P,1] combines + normalize + beta-add, so
    # every engine stays under the PE's 3.4us per q-tile.
    # Engine queues are IN-ORDER: an instruction waiting on a cross-engine
    # dependency head-of-line-blocks everything behind it.  The LN chain
    # has ~6 cross-engine hops, so it is staged as a skewed pipeline where
    # every instruction emitted in iteration i depends only on results
    # from iteration i-1 or older (except gamma, which trails its own
    # iteration's ts on the otherwise-idle GPSIMD).
    #   iter i:  mm+drain+sums(i) | combine+sqrt(i-1) | ts(i-2), gamma(i-2)
    #            | beta+store(i-3)
    # The normalize/gamma/beta chain runs IN PLACE over h (each stage is a
    # full pipeline step apart, so the RMW chain adds no new stalls); this
    # keeps the right-stack LN footprint small enough to coexist with at.
    RD = float(1.0 / D)
    st_h = {}
    st_sums = {}
    st_mu = {}
    st_rstd = {}

    def _stage_mm(qt):
        h = h_pool.tile([P, D], F32, tag="h", name=f"h{qt}")
        hsq = hsq_pool.tile([P, NC], F32, tag="hsq", name=f"hsq{qt}")
        sums = stat_pool.tile([P, 4], F32, tag="sums", name=f"sums{qt}")
        for dc in range(DCN):
            ps = ps_mm.tile([P, NC], F32, tag="mm", name=f"psO{qt}_{dc}")
            for v in range(DT):
                nc.tensor.matmul(
                    ps[:],
                    zt_sb[v][:, qt * P:(qt + 1) * P],
                    wvo_sb[v][:, dc * NC:(dc + 1) * NC],
                    start=(v == 0),
                    stop=(v == DT - 1),
                )
            nc.scalar.activation(
                h[:, dc * NC:(dc + 1) * NC], ps[:], AF.Copy,
                accum_out=sums[:, dc:dc + 1],
            )
            nc.scalar.activation(
                hsq[:], ps[:], AF.Square,
                accum_out=sums[:, 2 + dc:3 + dc],
            )
        st_h[qt] = h
        st_sums[qt] = sums

    def _stage_stats(qt):
        # mu = (s0+s1)/D; var = (q0+q1)/D - mu^2; rstd_pre = sqrt(var+eps)
        sums = st_sums[qt]
        mu = stat_pool.tile([P, 1], F32, tag="mu", name=f"mu{qt}")
        msum = stat_pool.tile([P, 1], F32, tag="msum", name=f"msum{qt}")
        nc.vector.tensor_tensor(msum[:], sums[:, 0:1], sums[:, 1:2], OP.add)
        nc.vector.tensor_scalar(
            out=mu[:], in0=msum[:], scalar1=RD, scalar2=None, op0=OP.mult,
        )
        m2 = stat_pool.tile([P, 1], F32, tag="m2", name=f"m2{qt}")
        nc.vector.tensor_tensor(m2[:], mu[:], mu[:], OP.mult)
        qsum = stat_pool.tile([P, 1], F32, tag="qsum", name=f"qsum{qt}")
        nc.vector.tensor_tensor(qsum[:], sums[:, 2:3], sums[:, 3:4], OP.add)
        var = stat_pool.tile([P, 1], F32, tag="var", name=f"var{qt}")
        nc.vector.scalar_tensor_tensor(
            var[:], qsum[:], RD, m2[:], OP.mult, OP.subtract,
        )
        rstd = stat_pool.tile([P, 1], F32, tag="rstd", name=f"rstd{qt}")
        nc.scalar.activation(rstd[:], var[:], AF.Sqrt, bias=eps_sb[:], scale=1.0)
        st_mu[qt] = mu
        st_rstd[qt] = rstd

    def _stage_norm(qt):
        # Column-halved ts/gamma: halves pipeline across DVE and GPSIMD,
        # halving the serial chain latency (matters for the last tiles).
        # Both write h in place.
        rstd = st_rstd[qt]
        nc.vector.reciprocal(rstd[:], rstd[:])
        h = st_h[qt]
        for c in range(2):
            cs = slice(c * NC, (c + 1) * NC)
            nc.vector.tensor_scalar(
                out=h[:, cs],
                in0=h[:, cs],
                scalar1=st_mu[qt][:],
                scalar2=rstd[:],
                op0=OP.subtract,
                op1=OP.mult,
            )
            nc.gpsimd.tensor_tensor(h[:, cs], h[:, cs], gamma_sb[:, cs], OP.mult)

    def _stage_store(qt):
        h = st_h[qt]
        for c in range(2):
            cs = slice(c * NC, (c + 1) * NC)
            nc.vector.tensor_tensor(h[:, cs], h[:, cs], beta_sb[:, cs], OP.add)
            nc.sync.dma_start(out[qt * P:(qt + 1) * P, cs], h[:, cs])

    # Interleave: O's first q-half runs right after C1's qc0 columns are
    # ready; C1's qc1 groups then cover the tail of those LN chains, so
    # only the last q-half's pipeline drains after the final matmul.
    def _o_batch(q0):
        for i in range(q0, q0 + 4):
            _stage_mm(i)
            if i - 1 >= q0:
                _stage_stats(i - 1)
            if i - 2 >= q0:
                _stage_norm(i - 2)
            if i - 3 >= q0:
                _stage_store(i - 3)

    _o_batch(0)
    leftovers = [lambda: _stage_stats(3), lambda: _stage_norm(2),
                 lambda: _stage_store(1), lambda: _stage_norm(3),
                 lambda: _stage_store(2), lambda: _stage_store(3)]
    for d in range(DT):
        _c1_group(d, 1)
        if leftovers:
            leftovers.pop(0)()
    for f in leftovers:
        f()
    at_pool.release()
    _o_batch(4)
    _stage_stats(7)
    _stage_norm(6)
    _stage_store(5)
    _stage_norm(7)
    _stage_store(6)
    _stage_store(7)
    stat_pool.release()
    hsq_pool.release()
    h_pool.release()
    gb_pool.release()
    xrow_pool.release()
    wvo_pool.release()
    zt_pool.release()


_PROGS: dict = {}


def _build_program(n_iters: int = 1, upto: str = "full",
                   loads_every_iter: bool = True):
    key = (n_iters, upto, loads_every_iter)
    if key not in _PROGS:
        nc = bacc.Bacc(
            "TRN2",
            target_bir_lowering=False,
            debug=False,
            enable_asserts=False,
            num_devices=N_CORES,
        )
        io = {
            "xb": nc.dram_tensor("xb", [S, D], BF16, kind="ExternalInput").ap(),
            "wqk": nc.dram_tensor("wqk", [D, D], BF16, kind="ExternalInput").ap(),
            "wvo": nc.dram_tensor("wvo", [D, D], BF16, kind="ExternalInput").ap(),
            "gamma_b": nc.dram_tensor("gamma_b", [P, D], F32, kind="ExternalInput").ap(),
            "beta_b": nc.dram_tensor("beta_b", [P, D], F32, kind="ExternalInput").ap(),
            "out": nc.dram_tensor("out", [NQ, D], F32, kind="ExternalOutput").ap(),
        }
        with tile.TileContext(nc) as tc:
            # Persistent constants: eps + ACT function-table pre-warm (Exp
            # for S, Sqrt/Square for O), once per program rather than per
            # iteration.
            const = tc.alloc_tile_pool(name="const", bufs=1, side="left")
            eps_sb = const.tile([P, 1], F32, tag="eps")
            nc.vector.memset(eps_sb[:], LN_EPS)
            warm = const.tile([P, 1], F32, tag="actwarm")
            nc.scalar.activation(warm[:], eps_sb[:], AF.Exp, scale=1.0)
            nc.scalar.activation(warm[:], eps_sb[:], AF.Sqrt, scale=1.0)
            nc.scalar.activation(warm[:], eps_sb[:], AF.Square, scale=1.0)
            io["eps_sb"] = eps_sb
            pre_pools = []
            if not loads_every_iter:
                # Diagnostic mode: inputs resident in SBUF, loaded once.
                nc_ = nc
                xrow_pool = tc.alloc_tile_pool(name="xrow", bufs=1, side="left")
                xr_sb = [xrow_pool.tile([P, D], BF16, tag=f"xr{st}", name=f"xr{st}") for st in range(ST)]
                gb_pool = tc.alloc_tile_pool(name="gbp", bufs=1, side="left")
                gamma_sb = gb_pool.tile([P, D], F32, tag="gamma", name="gamma_sb")
                beta_sb = gb_pool.tile([P, D], F32, tag="beta", name="beta_sb")
                xtb_pool = tc.alloc_tile_pool(name="xtb", bufs=1, side="left")
                xtb = [xtb_pool.tile([P, S], BF16, tag=f"xtb{d}", name=f"xtb{d}") for d in range(DT)]
                wqk_pool = tc.alloc_tile_pool(name="wqkp", bufs=1, side="left")
                wqk_sb = [wqk_pool.tile([P, D], BF16, tag=f"wqk{d}", name=f"wqk{d}") for d in range(DT)]
                wvo_pool = tc.alloc_tile_pool(name="wvop", bufs=1, side="right")
                wvo_sb = [wvo_pool.tile([P, D], BF16, tag=f"wvo{v}", name=f"wvo{v}") for v in range(DT)]
                pre_pools = [wvo_pool, wqk_pool, xtb_pool, gb_pool, xrow_pool]
                for sc in range(4):
                    for d in range(DT):
                        nc_.sync.dma_start(
                            xtb[d][:, sc * NC:(sc + 1) * NC],
                            io["xb"][sc * NC:(sc + 1) * NC, d * P:(d + 1) * P],
                            transpose=True,
                        )
                for d in range(DT):
                    nc_.sync.dma_start(wqk_sb[d][:], io["wqk"][d * P:(d + 1) * P, :])
                for st in range(ST):
                    nc_.sync.dma_start(xr_sb[st][:], io["xb"][st * P:(st + 1) * P, :])
                for v in range(DT):
                    nc_.sync.dma_start(wvo_sb[v][:], io["wvo"][v * P:(v + 1) * P, :])
                nc_.sync.dma_start(gamma_sb[:], io["gamma_b"][:])
                nc_.sync.dma_start(beta_sb[:], io["beta_b"][:])
                io["pre"] = {
                    "xr_sb": xr_sb, "xtb": xtb, "wqk_sb": wqk_sb,
                    "wvo_sb": wvo_sb, "gamma_sb": gamma_sb, "beta_sb": beta_sb,
                }
            for i in range(n_iters):
                with ExitStack() as ctx:
                    _emit(ctx, tc, io, upto)
            io.pop("pre", None)
            for p in pre_pools:
                p.release()
            const.release()
        nc.compile()
        _PROGS[key] = nc
    return _PROGS[key]


LAST_RESULTS = None


def _host_inputs(x, Wq, Wk, Wv, Wo, ln2_gamma, ln2_beta):
    """Fold weights and cast; returns the per-core input maps."""
    import ml_dtypes

    bf16 = ml_dtypes.bfloat16
    x = np.asarray(x, dtype=np.float32)
    Wq = np.asarray(Wq, dtype=np.float64)
    Wk = np.asarray(Wk, dtype=np.float64)
    Wv = np.asarray(Wv, dtype=np.float64)
    Wo = np.asarray(Wo, dtype=np.float64)
    wqk = np.ascontiguousarray((Wq @ Wk.T).astype(bf16))
    wvo = np.ascontiguousarray((Wv @ Wo).astype(bf16))
    gamma_b = np.ascontiguousarray(
        np.broadcast_to(np.asarray(ln2_gamma, dtype=np.float32), (P, D))
    )
    beta_b = np.ascontiguousarray(
        np.broadcast_to(np.asarray(ln2_beta, dtype=np.float32), (P, D))
    )
    x16 = x.astype(bf16)
    in_maps = []
    for c in range(N_CORES):
        b, h = c // 2, c % 2
        # Rotate so this core's query rows are rows 0:NQ.
        xb = np.ascontiguousarray(np.roll(x16[b], -h * NQ, axis=0))
        in_maps.append(
            {
                "xb": xb,
                "wqk": wqk,
                "wvo": wvo,
                "gamma_b": gamma_b,
                "beta_b": beta_b,
            }
        )
    return in_maps


def kernel(x, Wq, Wk, Wv, Wo, ln2_gamma, ln2_beta):
    global LAST_RESULTS
    in_maps = _host_inputs(x, Wq, Wk, Wv, Wo, ln2_gamma, ln2_beta)
    nc = _build_program()
    res = run_bass_kernel_spmd(nc, in_maps, list(range(N_CORES)))
    LAST_RESULTS = res
    out = np.empty((B, S, D), dtype=np.float32)
    for c in range(N_CORES):
        b, h = c // 2, c % 2
        out[b, h * NQ:(h + 1) * NQ] = res.results[c]["out"]
    return out

